# revision 2
# baseline (speedup 1.0000x reference)
"""Trainium2 Bass kernel for the DCN (modulated deformable conv) layer.

Self-contained: hardcodes all shapes. Shards data-parallel over (batch b x
row-half h) onto 8 NeuronCores; each core computes a [64, 64, 128] slab of
the [4, 64, 128, 128] output.

v3 over v2:
 - featT5 (x-partitioned, 5 x-shift slots) is built on the HOST and DMA'd
   directly as 5 slot loads from a [132, 64, 69] bf16 DRAM tensor --
   removes the on-device transpose+shift chain (~95us DVE stall).
 - om conv emitted in two row-halves interleaved with mask_build so the
   DVE starts ~25us earlier.
 - all bulk inputs land as bf16 in DRAM (half the DMA bytes); identity /
   conv weights pre-cast on host.
 - fvec (dynamic per-sample scale) applied to the static weight via one
   ACT-engine per-partition scale instead of a DVE tensor_scalar.
"""

import numpy as np
import ml_dtypes
from contextlib import ExitStack

import concourse.bass as bass
import concourse.bacc as bacc
import concourse.tile as tile
from concourse import mybir
from concourse.bass_utils import run_bass_kernel_spmd

F32 = mybir.dt.float32
BF16 = mybir.dt.bfloat16
ALU = mybir.AluOpType
ACTF = mybir.ActivationFunctionType

B, CIN, COUT, H, W, K = 4, 64, 64, 128, 128, 3
KK = K * K
NOUT = 64          # out rows per core
NR = 69            # feat rows resident per core (row idx = y + ky + sy + 1)
NRI = 66           # inter rows resident (conv halo 1)
BR = 16            # out rows per apply block
CLAMP = 0.999999
SH = CIN * NR      # featT5 shift-slot stride (elements)

_CACHED = {}


def _build_nc():
    nc = bacc.Bacc("TRN2", target_bir_lowering=False)

    # ---- DRAM I/O (per-core views; same program on all 8 cores) ----
    d_featT = nc.dram_tensor("featT", [132, CIN, NR], BF16, kind="ExternalInput")
    d_inter2 = nc.dram_tensor("inter2", [128, NRI, 130], BF16, kind="ExternalInput")
    d_w2 = nc.dram_tensor("w2", [128, 5, 64], BF16, kind="ExternalInput")
    d_comw2 = nc.dram_tensor("comw2", [128, 3, 27], BF16, kind="ExternalInput")
    d_comw1 = nc.dram_tensor("comw1", [CIN, 3, 27], BF16, kind="ExternalInput")
    d_comb = nc.dram_tensor("comb", [27, 1], F32, kind="ExternalInput")
    d_c1w = nc.dram_tensor("c1w", [128, 2, 128], BF16, kind="ExternalInput")
    d_fea = nc.dram_tensor("fea", [128, 2], BF16, kind="ExternalInput")
    d_bias2 = nc.dram_tensor("bias2", [COUT, 1], F32, kind="ExternalInput")
    d_ident = nc.dram_tensor("ident", [128, 128], BF16, kind="ExternalInput")
    d_out = nc.dram_tensor("out", [COUT, NOUT, W], F32, kind="ExternalOutput")

    with ExitStack() as ctx:
        tc = ctx.enter_context(tile.TileContext(nc))

        # ---------------- persistent pool ----------------
        pers = ctx.enter_context(tc.tile_pool(name="pers", bufs=1))
        identb = pers.tile([128, 128], BF16)
        w2b = pers.tile([128, 5, 64], BF16)
        comwb2 = pers.tile([128, 3, 27], BF16)
        comwb1 = pers.tile([CIN, 3, 27], BF16)
        combc = pers.tile([27, 1], F32)
        bias2c = pers.tile([COUT, 1], F32)
        omT = pers.tile([128, NOUT, 27], BF16)
        wys = pers.tile([128, 3, 3, 3, NOUT], BF16)   # [x; sy, kx, ky, y]
        wxs = pers.tile([128, 3, 3, 3, NOUT], BF16)   # [x; sx, kx, ky, y]
        m2 = pers.tile([128, 3, 3, 3, 3, NOUT], BF16)  # [x; sy, sx, kx, ky, y]
        featT5 = pers.tile([128, 5, CIN, NR], BF16)    # [x; shift, c, row]

        with tc.tile_pool(name="init", bufs=1) as initp:
            c1wb = initp.tile([128, 2, 128], BF16)
            feab = initp.tile([128, 2], BF16)
            fvec = initp.tile([128, 1], F32)
            w2raw = initp.tile([128, 5, 64], BF16)
            # small om-critical loads first on each queue
            nc.sync.dma_start(out=comwb2, in_=d_comw2[:, :, :])
            nc.sync.dma_start(out=comwb1, in_=d_comw1[:, :, :])
            nc.sync.dma_start(out=combc, in_=d_comb[:, :])
            nc.sync.dma_start(out=identb, in_=d_ident[:, :])
            nc.scalar.dma_start(out=c1wb, in_=d_c1w[:, :, :])
            nc.scalar.dma_start(out=feab, in_=d_fea[:, :])
            nc.scalar.dma_start(out=bias2c, in_=d_bias2[:, :])
            nc.scalar.dma_start(out=w2raw, in_=d_w2[:, :, :])
            # featT5: 5 direct slot loads (host-transposed), spread over queues
            for s, q in zip(range(5), [nc.sync, nc.sync, nc.scalar, nc.scalar,
                                       nc.sync]):
                q.dma_start(out=featT5[:, s, :, :], in_=d_featT[s : s + 128, :, :])
            # inter (om input) on the gpsimd queue
            interb2 = pers.tile([128, NRI, 130], BF16)
            for r0, r1 in [(0, 24), (24, 46), (46, NRI)]:
                nc.gpsimd.dma_start(out=interb2[:, r0:r1, :],
                                    in_=d_inter2[:, r0:r1, :])

            with tc.tile_pool(name="psum_sm", bufs=1, space="PSUM") as psum_sm:
                # fvec = c1_w @ fea -> [128, 1] (host duplicated cols -> both halves)
                ps_fv = psum_sm.tile([128, 1], F32)
                for k in range(2):
                    nc.tensor.matmul(ps_fv[:, :], c1wb[:, k, :], feab[:, k : k + 1],
                                     start=(k == 0), stop=(k == 1))
                nc.scalar.copy(fvec[:, :], ps_fv[:, :])
                # w2b = w2raw * fvec[c]  (ACT per-partition scale, bf16)
                nc.scalar.activation(w2b.rearrange("p a b -> p (a b)")[:, :],
                                     w2raw.rearrange("p a b -> p (a b)")[:, :],
                                     ACTF.Copy, scale=fvec[:, :])

            # ---------------- phase 1: om conv (tap-paired) ----------------
            with tc.tile_pool(name="omph", bufs=1) as omph:
                om_sb = omph.tile([27, NOUT, W], BF16)
                mbig = omph.tile([128, 8, KK, NOUT], F32)
                wmT = omph.tile([128, KK, NOUT], F32, tag="wm")
                w0T = omph.tile([128, KK, NOUT], F32, tag="w0")
                wpT = omph.tile([128, KK, NOUT], F32, tag="wp")

                def om_rows(lo, hi, ompsum):
                    for n in range(lo // 4, hi // 4):  # 512-wide chunks = 4 rows
                        ps = ompsum.tile([27, 512], F32)
                        y0 = 4 * n
                        for dy in range(3):
                            mv_pair = interb2[:, y0 + dy : y0 + dy + 4, 0:W]
                            nc.tensor.matmul(ps[:, :], comwb2[:, dy, :], mv_pair,
                                             start=(dy == 0), stop=False)
                            mv_sing = interb2[0:CIN, y0 + dy : y0 + dy + 4, 2 : 2 + W]
                            nc.tensor.matmul(ps[:, :], comwb1[:, dy, :], mv_sing,
                                             start=False, stop=(dy == 2))
                        nc.scalar.activation(
                            om_sb[:, y0 : y0 + 4, :].rearrange("p a b -> p (a b)"),
                            ps[:, :], ACTF.Identity, bias=combc[:, :])

                def om_transpose(lo, hi, ompsum):
                    for g in range(lo // 16, hi // 16):
                        pst = ompsum.tile([128, 16, 28], BF16)
                        for j in range(16):
                            y = 16 * g + j
                            nc.tensor.transpose(pst[:, j, 0:27],
                                                om_sb[:, y, :], identb[0:27, 0:27])
                        nc.scalar.copy(omT[:, 16 * g : 16 * (g + 1), :], pst[:, :, 0:27])

                def mask_build(h0, h1):
                    hl = h1 - h0
                    dyT, dxT, sgT, ey, ly, ay, f0, s = [
                        mbig[:, i, :, h0:h1] for i in range(8)]
                    # repack from omT (ch-minor) into [128, p, y] (ACT)
                    for dst, lo in [(dyT, 0), (dxT, 9), (sgT, 18)]:
                        nc.scalar.copy(dst,
                                       omT[:, h0:h1, lo : lo + 9].rearrange(
                                           "p y c -> p c y"))
                    nc.scalar.activation(sgT, sgT, ACTF.Sigmoid)

                    def wslot_ap(wt, si):
                        sl = wt[:, si, :, :, h0:h1]
                        return bass.AP(tensor=sl.tensor, offset=sl.offset,
                                       ap=[sl.ap[0], [64, 3], [192, 3], [1, hl]])

                    def pmaj(sl):
                        """Reorder a [128, 9(p), hl] view to dims (ky, kx, y)."""
                        return bass.AP(tensor=sl.tensor, offset=sl.offset,
                                       ap=[sl.ap[0], [192, 3], [64, 3], [1, hl]])

                    for dT, wt, fold_sig in [(dyT, wys, True), (dxT, wxs, False)]:
                        nc.vector.tensor_scalar(out=dT, in0=dT,
                                                scalar1=-CLAMP, scalar2=CLAMP,
                                                op0=ALU.max, op1=ALU.min)
                        nc.vector.tensor_scalar(out=ey, in0=dT, scalar1=0.0,
                                                scalar2=None, op0=ALU.is_lt)
                        nc.vector.tensor_tensor(out=ly, in0=dT, in1=ey, op=ALU.add)
                        nc.vector.tensor_scalar(out=ay, in0=ly, scalar1=-1.0,
                                                scalar2=1.0, op0=ALU.mult, op1=ALU.add)
                        nc.vector.tensor_scalar(out=f0, in0=ey, scalar1=-1.0,
                                                scalar2=1.0, op0=ALU.mult, op1=ALU.add)
                        wm_h = wmT[:, :, h0:h1]
                        w0_h = w0T[:, :, h0:h1]
                        wp_h = wpT[:, :, h0:h1]
                        nc.vector.tensor_tensor(out=wm_h, in0=ey, in1=ay, op=ALU.mult)
                        nc.vector.tensor_tensor(out=wp_h, in0=f0, in1=ly, op=ALU.mult)
                        nc.vector.tensor_tensor(out=s, in0=wm_h, in1=wp_h, op=ALU.add)
                        nc.vector.tensor_scalar(out=w0_h, in0=s, scalar1=-1.0,
                                                scalar2=1.0, op0=ALU.mult, op1=ALU.add)
                        for si, wk in enumerate([wm_h, w0_h, wp_h]):
                            if fold_sig:
                                nc.vector.tensor_tensor(out=wslot_ap(wt, si),
                                                        in0=pmaj(wk), in1=pmaj(sgT),
                                                        op=ALU.mult)
                            else:
                                nc.scalar.copy(wslot_ap(wt, si), pmaj(wk))

                    # m2[x; sy, sx, kx, ky, y] = wys[sy] * wxs[sx] (bf16, 2x)
                    for sy in range(3):
                        for sx in range(3):
                            osl = m2[:, sy, sx, :, :, h0:h1]
                            ysl = wys[:, sy, :, :, h0:h1]
                            y_b = bass.AP(tensor=ysl.tensor, offset=ysl.offset,
                                          ap=[ysl.ap[0], [192, 3], [64, 3],
                                              [1, hl]])
                            xsl = wxs[:, sx, :, :, h0:h1]
                            nc.vector.tensor_tensor(out=osl, in0=y_b, in1=xsl,
                                                    op=ALU.mult)

                with tc.tile_pool(name="ompsum", bufs=2, space="PSUM") as ompsum:
                    om_rows(0, 32, ompsum)
                    om_transpose(0, 32, ompsum)
                    mask_build(0, 32)
                    om_rows(32, 64, ompsum)
                    om_transpose(32, 64, ompsum)
                    mask_build(32, 64)

            # ---------------- phase 3: apply + back-transpose + einsum ----------
            SY_SX = [(sy, sx) for sy in range(3) for sx in range(3)]
            ft_full = featT5[:, :, :, :]

            with (
                tc.tile_pool(name="vpool", bufs=2) as vpool,
                tc.tile_pool(name="ppool", bufs=1) as ppool,
                tc.tile_pool(name="vblk", bufs=2) as vblk,
                tc.tile_pool(name="och", bufs=2) as och,
                tc.tile_pool(name="vpsum", bufs=2, space="PSUM") as vpsum,
            ):
                for nb in range(NOUT // BR):
                    y0 = BR * nb
                    vt = vpool.tile([128, 3, 3, CIN, BR], BF16, tag="vt")
                    A = ppool.tile([128, 3, 3, CIN, BR], BF16, tag="A")
                    val_blk = vblk.tile([128, 5, BR, W], BF16, tag="vb")
                    if nb < 2:
                        nc.vector.memset(val_blk[64:128, 4, :, :], 0.0)

                    # all 9 planes on DVE (3 free dims max per AP => one mult
                    # per kx).  The last block runs in two y-halves so the
                    # back-transpose + matmul tail starts earlier.
                    def dve_mult(dst, sy, sx, j0, j1):
                        for kx in range(3):
                            foff = (ft_full.offset + (sx + kx) * SH
                                    + (y0 + j0 + sy + 1))
                            fsl = bass.AP(tensor=ft_full.tensor, offset=foff,
                                          ap=[ft_full.ap[0], [1, 3], [NR, CIN],
                                              [1, j1 - j0]])
                            msl0 = m2[:, sy, sx, kx, :, y0 + j0 : y0 + j1]
                            msl = bass.AP(tensor=msl0.tensor, offset=msl0.offset,
                                          ap=[msl0.ap[0], msl0.ap[1], [0, CIN],
                                              msl0.ap[2]])
                            nc.vector.tensor_tensor(out=dst[:, kx, :, :, j0:j1],
                                                    in0=fsl, in1=msl, op=ALU.mult)

                    halves = [(0, BR)] if nb < NOUT // BR - 1 else [(0, 8), (8, BR)]
                    for j0, j1 in halves:
                        sy, sx = SY_SX[0]
                        dve_mult(vt, sy, sx, j0, j1)
                        for sy, sx in SY_SX[1:]:
                            dve_mult(A, sy, sx, j0, j1)
                            if j1 - j0 == BR:
                                nc.vector.tensor_tensor(out=vt[:, :, :, :, :],
                                                        in0=vt[:, :, :, :, :],
                                                        in1=A[:, :, :, :, :],
                                                        op=ALU.add)
                            else:
                                for kx in range(3):
                                    nc.vector.tensor_tensor(
                                        out=vt[:, kx, :, :, j0:j1],
                                        in0=vt[:, kx, :, :, j0:j1],
                                        in1=A[:, kx, :, :, j0:j1], op=ALU.add)

                    # back-transpose vt -> val_blk [(c, p-pair); t, y, x]
                    for t in range(5):
                        pst = vpsum.tile([128, BR * 128], BF16, tag="bt")
                        for pp in range(2):
                            p = 2 * t + pp
                            if p >= KK:
                                continue
                            ky, kx = p // 3, p % 3
                            for j in range(BR):
                                nc.tensor.transpose(
                                    pst[64 * pp : 64 * pp + 64, 128 * j : 128 * (j + 1)],
                                    vt[:, kx, ky, :, j], identb[:, :])
                        hi = 128 if t < 4 else 64
                        nc.scalar.copy(val_blk[0:hi, t, :, :], pst[0:hi, :])

                    oc = och.tile([COUT, BR, W], F32, tag="oc")
                    for c2 in range(BR // 4):
                        ps = vpsum.tile([COUT, 512], F32, tag="mm")
                        for t in range(5):
                            nc.tensor.matmul(ps[:, :], w2b[:, t, :],
                                             val_blk[:, t, 4 * c2 : 4 * c2 + 4, :],
                                             start=(t == 0), stop=(t == 4))
                        nc.scalar.activation(oc[:, 4 * c2 : 4 * c2 + 4, :], ps[:, :],
                                             ACTF.Identity, bias=bias2c[:, :])
                    nc.sync.dma_start(out=d_out[:, BR * nb : BR * nb + BR, :],
                                      in_=oc[:, :, :])

    nc.compile()
    return nc


def _host_prep(inputs):
    """Build the 8 per-core input maps (numpy marshalling only)."""
    bf = ml_dtypes.bfloat16
    feat = np.ascontiguousarray(inputs["input_feat"], dtype=np.float32)
    inter = np.ascontiguousarray(inputs["inter"], dtype=np.float32)
    fea = np.asarray(inputs["fea"], dtype=np.float32)[:, :, 0, 0]  # [B, 256]
    weight = np.asarray(inputs["weight"], dtype=np.float32)
    bias = np.asarray(inputs["bias"], dtype=np.float32)
    com_w = np.asarray(inputs["com_w"], dtype=np.float32)
    com_b = np.asarray(inputs["com_b"], dtype=np.float32)
    c1_w = np.asarray(inputs["c1_w"], dtype=np.float32)
    c2_w = np.asarray(inputs["c2_w"], dtype=np.float32)

    # fold c2 into the static weight:  weight2[o2, c, p] (parameter prep)
    w_r = weight.reshape(COUT, CIN, KK)
    weight2 = np.einsum("ao,ocp->acp", c2_w, w_r)  # [64, 64, 9]
    w2 = np.zeros((128, 5, 64), np.float32)  # [(c, p-pair), ktile, o2]
    for p in range(KK):
        t, pp = p // 2, p % 2
        w2[64 * pp : 64 * pp + 64, t, :] = weight2[:, :, p].T  # [c, o2]
    w2 = w2.astype(bf)
    bias2 = (c2_w @ bias).reshape(COUT, 1)

    # com_w reordered: channels [dy x9, dx x9, sig x9]; layout [cin, tap, 27]
    perm = list(range(0, 18, 2)) + list(range(1, 18, 2)) + list(range(18, 27))
    comw = np.ascontiguousarray(
        com_w[perm].reshape(27, CIN, KK).transpose(1, 2, 0))  # [CIN, KK, 27]
    comb = com_b[perm].reshape(27, 1).astype(np.float32)
    # tap pairing for om conv: kx=0/1 stacked in partition halves, kx=2 single
    comw2 = np.zeros((128, 3, 27), np.float32)
    comw1 = np.zeros((CIN, 3, 27), np.float32)
    for dy in range(3):
        comw2[0:CIN, dy] = comw[:, 3 * dy + 0]
        comw2[CIN:128, dy] = comw[:, 3 * dy + 1]
        comw1[:, dy] = comw[:, 3 * dy + 2]
    comw2 = comw2.astype(bf)
    comw1 = comw1.astype(bf)

    # c1w duplicated over output cols so the fvec matmul fills 128 partitions
    c1w = np.ascontiguousarray(c1_w.T.reshape(2, 128, COUT).transpose(1, 0, 2))
    c1w2 = np.concatenate([c1w, c1w], axis=2).astype(bf)  # [128, 2, 128]
    ident = np.eye(128, dtype=np.float32).astype(bf)

    in_maps = []
    for i in range(8):
        b, h = i // 2, i % 2
        r0 = NOUT * h
        # host-transposed feat: [x(132, padded +-2), c, row]
        fpadT = np.zeros((132, CIN, NR), bf)
        glo, ghi = r0 - 3, r0 - 3 + NR
        slo, shi = max(0, glo), min(H, ghi)
        fpadT[2 : 2 + W, :, slo - glo : shi - glo] = (
            feat[b, :, slo:shi, :].astype(bf).transpose(2, 0, 1))
        ipad2 = np.zeros((128, NRI, 130), np.float32)
        glo, ghi = r0 - 1, r0 - 1 + NRI
        slo, shi = max(0, glo), min(H, ghi)
        ipad2[0:CIN, slo - glo : shi - glo, 1 : 1 + W] = inter[b, :, slo:shi, :]
        ipad2[CIN:128, :, 0:129] = ipad2[0:CIN, :, 1:130]
        ipad2 = ipad2.astype(bf)
        feac = np.ascontiguousarray(fea[b].reshape(2, 128).T).astype(bf)
        in_maps.append(dict(featT=fpadT, inter2=ipad2, w2=w2, comw2=comw2,
                            comw1=comw1, comb=comb, c1w=c1w2, fea=feac,
                            bias2=bias2, ident=ident))
    return in_maps


def kernel(**inputs) -> np.ndarray:
    if "nc" not in _CACHED:
        _CACHED["nc"] = _build_nc()
    nc = _CACHED["nc"]
    in_maps = _host_prep(inputs)
    res = run_bass_kernel_spmd(nc, in_maps, core_ids=list(range(8)),
                               **_CACHED.get("run_kwargs", {}))
    _CACHED["last_result"] = res
    out = np.zeros((B, COUT, H, W), np.float32)
    for i in range(8):
        b, h = i // 2, i % 2
        out[b, :, NOUT * h : NOUT * (h + 1), :] = res.results[i]["out"]
    return out


# revision 9
# speedup vs baseline: 1.0279x; 1.0279x over previous
"""Trainium2 Bass kernel for the DCN (modulated deformable conv) layer.

Self-contained: hardcodes all shapes. Shards data-parallel over (batch b x
row-half h) onto 8 NeuronCores; each core computes a [64, 64, 128] slab of
the [4, 64, 128, 128] output.

v4 over v3:
 - om conv + mask build + apply interleaved at 16-row quarters: DVE starts
   mask(0,16) as soon as the first quarter of the om conv lands, and apply
   block 0 right after -- removes the ~36us DVE wait on the full om conv.
 - last 16 output rows run as two fully-pipelined 8-row blocks (their own
   back-transpose + matmul + store), halving the exposed tail.
 - val_blk is a manually rotated persistent double buffer; its tap-9
   upper-half zero fill happens once at init on the gpsimd queue.
v3 over v2:
 - featT5 (x-partitioned, 5 x-shift slots) built on the HOST, loaded as 5
   direct slot DMAs from a [132, 64, 69] bf16 DRAM tensor.
 - om conv emitted in row-halves; bulk inputs land as bf16; fvec applied
   to the static weight via one ACT-engine per-partition scale.
"""

import numpy as np
import ml_dtypes
from contextlib import ExitStack

import concourse.bass as bass
import concourse.bacc as bacc
import concourse.tile as tile
from concourse import mybir
from concourse.bass_utils import run_bass_kernel_spmd

F32 = mybir.dt.float32
BF16 = mybir.dt.bfloat16
ALU = mybir.AluOpType
ACTF = mybir.ActivationFunctionType

B, CIN, COUT, H, W, K = 4, 64, 64, 128, 128, 3
KK = K * K
NOUT = 64          # out rows per core
NR = 69            # feat rows resident per core (row idx = y + ky + sy + 1)
NRI = 66           # inter rows resident (conv halo 1)
QR = 16            # om/mask quarter rows
CLAMP = 0.999999
SH = CIN * NR      # featT5 shift-slot stride (elements)
BLOCKS = [(0, 16), (16, 16), (32, 16), (48, 8), (56, 8)]

_CACHED = {}


def _build_nc():
    nc = bacc.Bacc("TRN2", target_bir_lowering=False)

    # ---- DRAM I/O (per-core views; same program on all 8 cores) ----
    d_featT = nc.dram_tensor("featT", [132, CIN, NR], BF16, kind="ExternalInput")
    d_inter2 = nc.dram_tensor("inter2", [128, NRI, 130], BF16, kind="ExternalInput")
    d_w2 = nc.dram_tensor("w2", [128, 5, 64], BF16, kind="ExternalInput")
    d_comw2 = nc.dram_tensor("comw2", [128, 3, 27], BF16, kind="ExternalInput")
    d_comw1 = nc.dram_tensor("comw1", [CIN, 3, 27], BF16, kind="ExternalInput")
    d_comb = nc.dram_tensor("comb", [27, 1], F32, kind="ExternalInput")
    d_c1w = nc.dram_tensor("c1w", [128, 2, 128], BF16, kind="ExternalInput")
    d_fea = nc.dram_tensor("fea", [128, 2], BF16, kind="ExternalInput")
    d_bias2 = nc.dram_tensor("bias2", [COUT, 1], F32, kind="ExternalInput")
    d_ident = nc.dram_tensor("ident", [128, 128], BF16, kind="ExternalInput")
    d_out = nc.dram_tensor("out", [COUT, NOUT, W], F32, kind="ExternalOutput")

    with ExitStack() as ctx:
        tc = ctx.enter_context(tile.TileContext(nc))

        # ---------------- persistent pool ----------------
        pers = ctx.enter_context(tc.tile_pool(name="pers", bufs=1))
        identb = pers.tile([128, 128], BF16)
        w2b = pers.tile([128, 5, 64], BF16)
        bias2c = pers.tile([COUT, 1], F32)
        omT = pers.tile([128, NOUT, 27], BF16)
        wys = pers.tile([128, 3, 3, 3, NOUT], BF16)   # [x; sy, kx, ky, y]
        wxs = pers.tile([128, 3, 3, 3, NOUT], BF16)   # [x; sx, kx, ky, y]
        m2 = pers.tile([128, 3, 3, 3, 3, NOUT], BF16)  # [x; sy, sx, kx, ky, y]
        featT5 = pers.tile([128, 5, CIN, NR], BF16)    # [x; shift, c, row]
        # om/mask per-quarter scratch
        mbig = pers.tile([128, 8, KK, QR], F32)
        wmT = pers.tile([128, KK, QR], F32, tag="wm")
        w0T = pers.tile([128, KK, QR], F32, tag="w0")
        wpT = pers.tile([128, KK, QR], F32, tag="wp")
        # apply: manually rotated val_blk double buffer
        val_blk = pers.tile([128, 2, 5, 16, W], BF16)

        # ---------------- init loads ----------------
        omsb_cm = tc.tile_pool(name="omsb", bufs=1)
        omsb = omsb_cm.__enter__()
        comwb2 = omsb.tile([128, 3, 27], BF16)
        comwb1 = omsb.tile([CIN, 3, 27], BF16)
        combc = omsb.tile([27, 1], F32)
        c1wb = omsb.tile([128, 2, 128], BF16)
        feab = omsb.tile([128, 2], BF16)
        fvec = omsb.tile([128, 1], F32)
        w2raw = omsb.tile([128, 5, 64], BF16)
        interb2 = omsb.tile([128, NRI, 130], BF16)
        om_sb = omsb.tile([27, QR, W], BF16)
        nc.sync.dma_start(out=comwb2, in_=d_comw2[:, :, :])
        nc.sync.dma_start(out=comwb1, in_=d_comw1[:, :, :])
        nc.sync.dma_start(out=combc, in_=d_comb[:, :])
        nc.sync.dma_start(out=identb, in_=d_ident[:, :])
        nc.scalar.dma_start(out=c1wb, in_=d_c1w[:, :, :])
        nc.scalar.dma_start(out=feab, in_=d_fea[:, :])
        nc.scalar.dma_start(out=bias2c, in_=d_bias2[:, :])
        nc.scalar.dma_start(out=w2raw, in_=d_w2[:, :, :])
        for s, q in zip(range(5), [nc.sync, nc.sync, nc.scalar, nc.scalar,
                                   nc.sync]):
            q.dma_start(out=featT5[:, s, :, :], in_=d_featT[s : s + 128, :, :])
        # zero tap-9 upper half of both val_blk buffers once
        nc.gpsimd.memset(val_blk[64:128, :, 4, :, :], 0.0)

        with tc.tile_pool(name="psum_sm", bufs=1, space="PSUM") as psum_sm:
            # fvec = c1_w @ fea -> [128, 1] (host duplicated cols -> both halves)
            ps_fv = psum_sm.tile([128, 1], F32)
            for k in range(2):
                nc.tensor.matmul(ps_fv[:, :], c1wb[:, k, :], feab[:, k : k + 1],
                                 start=(k == 0), stop=(k == 1))
            nc.scalar.copy(fvec[:, :], ps_fv[:, :])
            # w2b = w2raw * fvec[c]  (ACT per-partition scale, bf16)
            nc.scalar.activation(w2b.rearrange("p a b -> p (a b)")[:, :],
                                 w2raw.rearrange("p a b -> p (a b)")[:, :],
                                 ACTF.Copy, scale=fvec[:, :])

        # ---------------- om conv quarter + mask build ----------------
        def om_quarter(q, ompsum, interb2, om_sb):
            yq = QR * q
            for n in range(QR // 4):  # 512-wide chunks = 4 rows
                ps = ompsum.tile([27, 512], F32)
                yl = 4 * n
                for dy in range(3):
                    mv_pair = interb2[:, yq + yl + dy : yq + yl + dy + 4, 0:W]
                    nc.tensor.matmul(ps[:, :], comwb2[:, dy, :], mv_pair,
                                     start=(dy == 0), stop=False)
                    mv_sing = interb2[0:CIN, yq + yl + dy : yq + yl + dy + 4,
                                      2 : 2 + W]
                    nc.tensor.matmul(ps[:, :], comwb1[:, dy, :], mv_sing,
                                     start=False, stop=(dy == 2))
                nc.scalar.activation(
                    om_sb[:, yl : yl + 4, :].rearrange("p a b -> p (a b)"),
                    ps[:, :], ACTF.Identity, bias=combc[:, :])
            pst = ompsum.tile([128, QR, 28], BF16)
            for j in range(QR):
                nc.tensor.transpose(pst[:, j, 0:27], om_sb[:, j, :],
                                    identb[0:27, 0:27])
            nc.scalar.copy(omT[:, yq : yq + QR, :], pst[:, :, 0:27])

        def mask_build(q):
            h0, h1 = QR * q, QR * q + QR
            dyT, dxT, sgT, ey, ly, ay, f0, s = [
                mbig[:, i, :, :] for i in range(8)]
            # repack from omT (ch-minor) into [128, p, y] (ACT)
            for dst, lo in [(dyT, 0), (dxT, 9), (sgT, 18)]:
                nc.scalar.copy(dst,
                               omT[:, h0:h1, lo : lo + 9].rearrange(
                                   "p y c -> p c y"))
            nc.scalar.activation(sgT, sgT, ACTF.Sigmoid)

            def wslot_ap(wt, si):
                sl = wt[:, si, :, :, h0:h1]
                return bass.AP(tensor=sl.tensor, offset=sl.offset,
                               ap=[sl.ap[0], [64, 3], [192, 3], [1, QR]])

            def pmaj(sl):
                """Reorder a [128, 9(p), QR] scratch view to (ky, kx, y)."""
                return bass.AP(tensor=sl.tensor, offset=sl.offset,
                               ap=[sl.ap[0], [3 * QR, 3], [QR, 3], [1, QR]])

            for dT, wt, fold_sig in [(dyT, wys, True), (dxT, wxs, False)]:
                nc.vector.tensor_scalar(out=dT, in0=dT,
                                        scalar1=-CLAMP, scalar2=CLAMP,
                                        op0=ALU.max, op1=ALU.min)
                nc.vector.tensor_scalar(out=ey, in0=dT, scalar1=0.0,
                                        scalar2=None, op0=ALU.is_lt)
                nc.vector.tensor_tensor(out=ly, in0=dT, in1=ey, op=ALU.add)
                nc.vector.tensor_scalar(out=ay, in0=ly, scalar1=-1.0,
                                        scalar2=1.0, op0=ALU.mult, op1=ALU.add)
                nc.vector.tensor_scalar(out=f0, in0=ey, scalar1=-1.0,
                                        scalar2=1.0, op0=ALU.mult, op1=ALU.add)
                nc.vector.tensor_tensor(out=wmT[:, :, :], in0=ey, in1=ay,
                                        op=ALU.mult)
                nc.vector.tensor_tensor(out=wpT[:, :, :], in0=f0, in1=ly,
                                        op=ALU.mult)
                nc.vector.tensor_tensor(out=s, in0=wmT[:, :, :],
                                        in1=wpT[:, :, :], op=ALU.add)
                nc.vector.tensor_scalar(out=w0T[:, :, :], in0=s, scalar1=-1.0,
                                        scalar2=1.0, op0=ALU.mult, op1=ALU.add)
                for si, wk in enumerate([wmT, w0T, wpT]):
                    if fold_sig:
                        nc.vector.tensor_tensor(out=wslot_ap(wt, si),
                                                in0=pmaj(wk[:, :, :]),
                                                in1=pmaj(sgT), op=ALU.mult)
                    else:
                        nc.scalar.copy(wslot_ap(wt, si), pmaj(wk[:, :, :]))

            # m2[x; sy, sx, kx, ky, y] = wys[sy] * wxs[sx] (bf16, 2x)
            for sy in range(3):
                for sx in range(3):
                    osl = m2[:, sy, sx, :, :, h0:h1]
                    ysl = wys[:, sy, :, :, h0:h1]
                    y_b = bass.AP(tensor=ysl.tensor, offset=ysl.offset,
                                  ap=[ysl.ap[0], [192, 3], [64, 3], [1, QR]])
                    xsl = wxs[:, sx, :, :, h0:h1]
                    nc.vector.tensor_tensor(out=osl, in0=y_b, in1=xsl,
                                            op=ALU.mult)

        # ---------------- apply block ----------------
        SY_SX = [(sy, sx) for sy in range(3) for sx in range(3)]
        ft_full = featT5[:, :, :, :]

        def apply_block(bi, y0, rows, vpool, ppool, och, vpsum):
            vt = vpool.tile([128, 3, 3, CIN, rows], BF16, tag=f"vt{rows}")
            A = ppool.tile([128, 3, 3, CIN, rows], BF16, tag=f"A{rows}")
            vb = val_blk[:, bi % 2, :, 0:rows, :]

            def dve_mult(dst, sy, sx):
                for kx in range(3):
                    foff = (ft_full.offset + (sx + kx) * SH + (y0 + sy + 1))
                    fsl = bass.AP(tensor=ft_full.tensor, offset=foff,
                                  ap=[ft_full.ap[0], [1, 3], [NR, CIN],
                                      [1, rows]])
                    msl0 = m2[:, sy, sx, kx, :, y0 : y0 + rows]
                    msl = bass.AP(tensor=msl0.tensor, offset=msl0.offset,
                                  ap=[msl0.ap[0], msl0.ap[1], [0, CIN],
                                      msl0.ap[2]])
                    nc.vector.tensor_tensor(out=dst[:, kx, :, :, :],
                                            in0=fsl, in1=msl, op=ALU.mult)

            sy, sx = SY_SX[0]
            dve_mult(vt, sy, sx)
            for sy, sx in SY_SX[1:]:
                dve_mult(A, sy, sx)
                nc.vector.tensor_tensor(out=vt[:, :, :, :, :],
                                        in0=vt[:, :, :, :, :],
                                        in1=A[:, :, :, :, :], op=ALU.add)

            # back-transpose vt -> val_blk [(c, p-pair); t, y, x]
            for t in range(5):
                pst = vpsum.tile([128, rows * 128], BF16, tag="bt")
                for pp in range(2):
                    p = 2 * t + pp
                    if p >= KK:
                        continue
                    ky, kx = p // 3, p % 3
                    for j in range(rows):
                        nc.tensor.transpose(
                            pst[64 * pp : 64 * pp + 64, 128 * j : 128 * (j + 1)],
                            vt[:, kx, ky, :, j], identb[:, :])
                hi = 128 if t < 4 else 64
                nc.scalar.copy(vb[0:hi, t, :, :], pst[0:hi, :])

            oc = och.tile([COUT, 16, W], F32, tag="oc")
            for c2 in range(rows // 4):
                ps = vpsum.tile([COUT, 512], F32, tag="mm")
                for t in range(5):
                    nc.tensor.matmul(ps[:, :], w2b[:, t, :],
                                     vb[:, t, 4 * c2 : 4 * c2 + 4, :],
                                     start=(t == 0), stop=(t == 4))
                nc.scalar.activation(oc[:, 4 * c2 : 4 * c2 + 4, :], ps[:, :],
                                     ACTF.Identity, bias=bias2c[:, :])
            nc.sync.dma_start(out=d_out[:, y0 : y0 + rows, :],
                              in_=oc[:, 0:rows, :])

        # ---------------- schedule ----------------
        with tc.tile_pool(name="ompsum", bufs=2, space="PSUM") as ompsum:
            for r0, r1 in [(0, 24), (24, 46), (46, NRI)]:
                nc.gpsimd.dma_start(out=interb2[:, r0:r1, :],
                                    in_=d_inter2[:, r0:r1, :])
            om_quarter(0, ompsum, interb2, om_sb)
            mask_build(0)
            om_quarter(1, ompsum, interb2, om_sb)
            om_quarter(2, ompsum, interb2, om_sb)
            om_quarter(3, ompsum, interb2, om_sb)
        omsb_cm.__exit__(None, None, None)

        with (
            tc.tile_pool(name="vpool", bufs=2) as vpool,
            tc.tile_pool(name="ppool", bufs=1) as ppool,
            tc.tile_pool(name="och", bufs=2) as och,
            tc.tile_pool(name="vpsum", bufs=2, space="PSUM") as vpsum,
        ):
            for bi, (y0, rows) in enumerate(BLOCKS):
                apply_block(bi, y0, rows, vpool, ppool, och, vpsum)
                nxt = 1 + bi
                if nxt < 4 and BLOCKS[bi + 1][0] // QR == nxt:
                    mask_build(nxt)

    nc.compile()
    return nc


def _host_prep(inputs):
    """Build the 8 per-core input maps (numpy marshalling only)."""
    bf = ml_dtypes.bfloat16
    feat = np.ascontiguousarray(inputs["input_feat"], dtype=np.float32)
    inter = np.ascontiguousarray(inputs["inter"], dtype=np.float32)
    fea = np.asarray(inputs["fea"], dtype=np.float32)[:, :, 0, 0]  # [B, 256]
    weight = np.asarray(inputs["weight"], dtype=np.float32)
    bias = np.asarray(inputs["bias"], dtype=np.float32)
    com_w = np.asarray(inputs["com_w"], dtype=np.float32)
    com_b = np.asarray(inputs["com_b"], dtype=np.float32)
    c1_w = np.asarray(inputs["c1_w"], dtype=np.float32)
    c2_w = np.asarray(inputs["c2_w"], dtype=np.float32)

    # fold c2 into the static weight:  weight2[o2, c, p] (parameter prep)
    w_r = weight.reshape(COUT, CIN, KK)
    weight2 = np.einsum("ao,ocp->acp", c2_w, w_r)  # [64, 64, 9]
    w2 = np.zeros((128, 5, 64), np.float32)  # [(c, p-pair), ktile, o2]
    for p in range(KK):
        t, pp = p // 2, p % 2
        w2[64 * pp : 64 * pp + 64, t, :] = weight2[:, :, p].T  # [c, o2]
    w2 = w2.astype(bf)
    bias2 = (c2_w @ bias).reshape(COUT, 1)

    # com_w reordered: channels [dy x9, dx x9, sig x9]; layout [cin, tap, 27]
    perm = list(range(0, 18, 2)) + list(range(1, 18, 2)) + list(range(18, 27))
    comw = np.ascontiguousarray(
        com_w[perm].reshape(27, CIN, KK).transpose(1, 2, 0))  # [CIN, KK, 27]
    comb = com_b[perm].reshape(27, 1).astype(np.float32)
    # tap pairing for om conv: kx=0/1 stacked in partition halves, kx=2 single
    comw2 = np.zeros((128, 3, 27), np.float32)
    comw1 = np.zeros((CIN, 3, 27), np.float32)
    for dy in range(3):
        comw2[0:CIN, dy] = comw[:, 3 * dy + 0]
        comw2[CIN:128, dy] = comw[:, 3 * dy + 1]
        comw1[:, dy] = comw[:, 3 * dy + 2]
    comw2 = comw2.astype(bf)
    comw1 = comw1.astype(bf)

    # c1w duplicated over output cols so the fvec matmul fills 128 partitions
    c1w = np.ascontiguousarray(c1_w.T.reshape(2, 128, COUT).transpose(1, 0, 2))
    c1w2 = np.concatenate([c1w, c1w], axis=2).astype(bf)  # [128, 2, 128]
    ident = np.eye(128, dtype=np.float32).astype(bf)

    in_maps = []
    for i in range(8):
        b, h = i // 2, i % 2
        r0 = NOUT * h
        # host-transposed feat: [x(132, padded +-2), c, row]
        fpadT = np.zeros((132, CIN, NR), bf)
        glo, ghi = r0 - 3, r0 - 3 + NR
        slo, shi = max(0, glo), min(H, ghi)
        fpadT[2 : 2 + W, :, slo - glo : shi - glo] = (
            feat[b, :, slo:shi, :].astype(bf).transpose(2, 0, 1))
        ipad2 = np.zeros((128, NRI, 130), np.float32)
        glo, ghi = r0 - 1, r0 - 1 + NRI
        slo, shi = max(0, glo), min(H, ghi)
        ipad2[0:CIN, slo - glo : shi - glo, 1 : 1 + W] = inter[b, :, slo:shi, :]
        ipad2[CIN:128, :, 0:129] = ipad2[0:CIN, :, 1:130]
        ipad2 = ipad2.astype(bf)
        feac = np.ascontiguousarray(fea[b].reshape(2, 128).T).astype(bf)
        in_maps.append(dict(featT=fpadT, inter2=ipad2, w2=w2, comw2=comw2,
                            comw1=comw1, comb=comb, c1w=c1w2, fea=feac,
                            bias2=bias2, ident=ident))
    return in_maps


def kernel(**inputs) -> np.ndarray:
    if "nc" not in _CACHED:
        _CACHED["nc"] = _build_nc()
    nc = _CACHED["nc"]
    in_maps = _host_prep(inputs)
    res = run_bass_kernel_spmd(nc, in_maps, core_ids=list(range(8)),
                               **_CACHED.get("run_kwargs", {}))
    _CACHED["last_result"] = res
    out = np.zeros((B, COUT, H, W), np.float32)
    for i in range(8):
        b, h = i // 2, i % 2
        out[b, :, NOUT * h : NOUT * (h + 1), :] = res.results[i]["out"]
    return out


# revision 11
# speedup vs baseline: 1.0398x; 1.0116x over previous
"""Trainium2 Bass kernel for the DCN (modulated deformable conv) layer.

Self-contained: hardcodes all shapes. Shards data-parallel over (batch b x
row-half h) onto 8 NeuronCores; each core computes a [64, 64, 128] slab of
the [4, 64, 128, 128] output.

v4 over v3:
 - om conv + mask build + apply interleaved at 16-row quarters: DVE starts
   mask(0,16) as soon as the first quarter of the om conv lands, and apply
   block 0 right after -- removes the ~36us DVE wait on the full om conv.
 - last 16 output rows run as two fully-pipelined 8-row blocks (their own
   back-transpose + matmul + store), halving the exposed tail.
 - val_blk is a manually rotated persistent double buffer; its tap-9
   upper-half zero fill happens once at init on the gpsimd queue.
v3 over v2:
 - featT5 (x-partitioned, 5 x-shift slots) built on the HOST, loaded as 5
   direct slot DMAs from a [132, 64, 69] bf16 DRAM tensor.
 - om conv emitted in row-halves; bulk inputs land as bf16; fvec applied
   to the static weight via one ACT-engine per-partition scale.
"""

import numpy as np
import ml_dtypes
from contextlib import ExitStack

import concourse.bass as bass
import concourse.bacc as bacc
import concourse.tile as tile
from concourse import mybir
from concourse.bass_utils import run_bass_kernel_spmd

F32 = mybir.dt.float32
BF16 = mybir.dt.bfloat16
ALU = mybir.AluOpType
ACTF = mybir.ActivationFunctionType

B, CIN, COUT, H, W, K = 4, 64, 64, 128, 128, 3
KK = K * K
NOUT = 64          # out rows per core
NR = 69            # feat rows resident per core (row idx = y + ky + sy + 1)
NRI = 66           # inter rows resident (conv halo 1)
QR = 32            # om/mask half rows
CLAMP = 0.999999
SH = CIN * NR      # featT5 shift-slot stride (elements)
BLOCKS = [(0, 16), (16, 16), (32, 16), (48, 8), (56, 8)]

_CACHED = {}


def _build_nc():
    nc = bacc.Bacc("TRN2", target_bir_lowering=False)

    # ---- DRAM I/O (per-core views; same program on all 8 cores) ----
    d_featT = nc.dram_tensor("featT", [132, CIN, NR], BF16, kind="ExternalInput")
    d_inter2 = nc.dram_tensor("inter2", [128, NRI, 130], BF16, kind="ExternalInput")
    d_w2 = nc.dram_tensor("w2", [128, 5, 64], BF16, kind="ExternalInput")
    d_comw2 = nc.dram_tensor("comw2", [128, 3, 27], BF16, kind="ExternalInput")
    d_comw1 = nc.dram_tensor("comw1", [CIN, 3, 27], BF16, kind="ExternalInput")
    d_comb = nc.dram_tensor("comb", [27, 1], F32, kind="ExternalInput")
    d_c1w = nc.dram_tensor("c1w", [128, 2, 128], BF16, kind="ExternalInput")
    d_fea = nc.dram_tensor("fea", [128, 2], BF16, kind="ExternalInput")
    d_bias2 = nc.dram_tensor("bias2", [COUT, 1], F32, kind="ExternalInput")
    d_ident = nc.dram_tensor("ident", [128, 128], BF16, kind="ExternalInput")
    d_out = nc.dram_tensor("out", [COUT, NOUT, W], F32, kind="ExternalOutput")

    with ExitStack() as ctx:
        tc = ctx.enter_context(tile.TileContext(nc))

        # ---------------- persistent pool ----------------
        pers = ctx.enter_context(tc.tile_pool(name="pers", bufs=1))
        identb = pers.tile([128, 128], BF16)
        w2b = pers.tile([128, 5, 64], BF16)
        bias2c = pers.tile([COUT, 1], F32)
        omT = pers.tile([128, NOUT, 27], BF16)
        wys = pers.tile([128, 3, 3, 3, NOUT], BF16)   # [x; sy, kx, ky, y]
        wxs = pers.tile([128, 3, 3, 3, NOUT], BF16)   # [x; sx, kx, ky, y]
        m2 = pers.tile([128, 3, 3, 3, 3, NOUT], BF16)  # [x; sy, sx, kx, ky, y]
        featT5 = pers.tile([128, 5, CIN, NR], BF16)    # [x; shift, c, row]
        # om/mask per-quarter scratch
        mbig = pers.tile([128, 8, KK, QR], BF16)
        wmT = pers.tile([128, KK, QR], BF16, tag="wm")
        w0T = pers.tile([128, KK, QR], BF16, tag="w0")
        wpT = pers.tile([128, KK, QR], BF16, tag="wp")
        # apply: manually rotated val_blk double buffer
        val_blk = pers.tile([128, 2, 5, 16, W], BF16)

        # ---------------- init loads ----------------
        omsb_cm = tc.tile_pool(name="omsb", bufs=1)
        omsb = omsb_cm.__enter__()
        comwb2 = omsb.tile([128, 3, 27], BF16)
        comwb1 = omsb.tile([CIN, 3, 27], BF16)
        combc = omsb.tile([27, 1], F32)
        c1wb = omsb.tile([128, 2, 128], BF16)
        feab = omsb.tile([128, 2], BF16)
        fvec = omsb.tile([128, 1], F32)
        w2raw = omsb.tile([128, 5, 64], BF16)
        interb2 = omsb.tile([128, NRI, 130], BF16)
        om_sb = omsb.tile([27, QR, W], BF16)
        nc.sync.dma_start(out=comwb2, in_=d_comw2[:, :, :])
        nc.sync.dma_start(out=comwb1, in_=d_comw1[:, :, :])
        nc.sync.dma_start(out=combc, in_=d_comb[:, :])
        nc.sync.dma_start(out=identb, in_=d_ident[:, :])
        nc.scalar.dma_start(out=c1wb, in_=d_c1w[:, :, :])
        nc.scalar.dma_start(out=feab, in_=d_fea[:, :])
        nc.scalar.dma_start(out=bias2c, in_=d_bias2[:, :])
        nc.scalar.dma_start(out=w2raw, in_=d_w2[:, :, :])
        for s, q in zip(range(5), [nc.sync, nc.sync, nc.scalar, nc.scalar,
                                   nc.sync]):
            q.dma_start(out=featT5[:, s, :, :], in_=d_featT[s : s + 128, :, :])
        # zero tap-9 upper half of both val_blk buffers once
        nc.gpsimd.memset(val_blk[64:128, :, 4, :, :], 0.0)

        # PE p-state warmup: keep the array busy while the inter DMA lands
        with tc.tile_pool(name="warm", bufs=1, space="PSUM") as warmp:
            ps_w = warmp.tile([128, 128], F32)
            for _ in range(24):
                nc.tensor.matmul(ps_w[:, :], identb[:, :], identb[:, :],
                                 start=True, stop=True)

        with tc.tile_pool(name="psum_sm", bufs=1, space="PSUM") as psum_sm:
            # fvec = c1_w @ fea -> [128, 1] (host duplicated cols -> both halves)
            ps_fv = psum_sm.tile([128, 1], F32)
            for k in range(2):
                nc.tensor.matmul(ps_fv[:, :], c1wb[:, k, :], feab[:, k : k + 1],
                                 start=(k == 0), stop=(k == 1))
            nc.scalar.copy(fvec[:, :], ps_fv[:, :])
            # w2b = w2raw * fvec[c]  (ACT per-partition scale, bf16)
            nc.scalar.activation(w2b.rearrange("p a b -> p (a b)")[:, :],
                                 w2raw.rearrange("p a b -> p (a b)")[:, :],
                                 ACTF.Copy, scale=fvec[:, :])

        # ---------------- om conv quarter + mask build ----------------
        def om_quarter(q, ompsum, interb2, om_sb):
            yq = QR * q
            for n in range(QR // 4):  # 512-wide chunks = 4 rows
                ps = ompsum.tile([27, 512], F32)
                yl = 4 * n
                for dy in range(3):
                    mv_pair = interb2[:, yq + yl + dy : yq + yl + dy + 4, 0:W]
                    nc.tensor.matmul(ps[:, :], comwb2[:, dy, :], mv_pair,
                                     start=(dy == 0), stop=False)
                    mv_sing = interb2[0:CIN, yq + yl + dy : yq + yl + dy + 4,
                                      2 : 2 + W]
                    nc.tensor.matmul(ps[:, :], comwb1[:, dy, :], mv_sing,
                                     start=False, stop=(dy == 2))
                nc.scalar.activation(
                    om_sb[:, yl : yl + 4, :].rearrange("p a b -> p (a b)"),
                    ps[:, :], ACTF.Identity, bias=combc[:, :])
            for g in range(QR // 16):
                pst = ompsum.tile([128, 16, 28], BF16)
                for j in range(16):
                    nc.tensor.transpose(pst[:, j, 0:27],
                                        om_sb[:, 16 * g + j, :],
                                        identb[0:27, 0:27])
                nc.scalar.copy(omT[:, yq + 16 * g : yq + 16 * g + 16, :],
                               pst[:, :, 0:27])

        def mask_build(q):
            h0, h1 = QR * q, QR * q + QR
            dyT, dxT, sgT, ey, ly, ay, f0, s = [
                mbig[:, i, :, :] for i in range(8)]
            # repack from omT (ch-minor) into [128, p, y] (ACT)
            for dst, lo in [(dyT, 0), (dxT, 9), (sgT, 18)]:
                nc.scalar.copy(dst,
                               omT[:, h0:h1, lo : lo + 9].rearrange(
                                   "p y c -> p c y"))
            nc.scalar.activation(sgT, sgT, ACTF.Sigmoid)

            def wslot_ap(wt, si):
                sl = wt[:, si, :, :, h0:h1]
                return bass.AP(tensor=sl.tensor, offset=sl.offset,
                               ap=[sl.ap[0], [64, 3], [192, 3], [1, QR]])

            def pmaj(sl):
                """Reorder a [128, 9(p), QR] scratch view to (ky, kx, y)."""
                return bass.AP(tensor=sl.tensor, offset=sl.offset,
                               ap=[sl.ap[0], [3 * QR, 3], [QR, 3], [1, QR]])

            for dT, wt, fold_sig in [(dyT, wys, True), (dxT, wxs, False)]:
                nc.vector.tensor_scalar(out=dT, in0=dT,
                                        scalar1=-CLAMP, scalar2=CLAMP,
                                        op0=ALU.max, op1=ALU.min)
                nc.vector.tensor_scalar(out=ey, in0=dT, scalar1=0.0,
                                        scalar2=None, op0=ALU.is_lt)
                nc.vector.tensor_tensor(out=ly, in0=dT, in1=ey, op=ALU.add)
                nc.vector.tensor_scalar(out=ay, in0=ly, scalar1=-1.0,
                                        scalar2=1.0, op0=ALU.mult, op1=ALU.add)
                nc.vector.tensor_scalar(out=f0, in0=ey, scalar1=-1.0,
                                        scalar2=1.0, op0=ALU.mult, op1=ALU.add)
                nc.vector.tensor_tensor(out=wmT[:, :, :], in0=ey, in1=ay,
                                        op=ALU.mult)
                nc.vector.tensor_tensor(out=wpT[:, :, :], in0=f0, in1=ly,
                                        op=ALU.mult)
                nc.vector.tensor_tensor(out=s, in0=wmT[:, :, :],
                                        in1=wpT[:, :, :], op=ALU.add)
                nc.vector.tensor_scalar(out=w0T[:, :, :], in0=s, scalar1=-1.0,
                                        scalar2=1.0, op0=ALU.mult, op1=ALU.add)
                for si, wk in enumerate([wmT, w0T, wpT]):
                    if fold_sig:
                        nc.vector.tensor_tensor(out=wslot_ap(wt, si),
                                                in0=pmaj(wk[:, :, :]),
                                                in1=pmaj(sgT), op=ALU.mult)
                    else:
                        nc.scalar.copy(wslot_ap(wt, si), pmaj(wk[:, :, :]))

            # m2[x; sy, sx, kx, ky, y] = wys[sy] * wxs[sx] (bf16, 2x)
            for sy in range(3):
                for sx in range(3):
                    osl = m2[:, sy, sx, :, :, h0:h1]
                    ysl = wys[:, sy, :, :, h0:h1]
                    y_b = bass.AP(tensor=ysl.tensor, offset=ysl.offset,
                                  ap=[ysl.ap[0], [192, 3], [64, 3], [1, QR]])
                    xsl = wxs[:, sx, :, :, h0:h1]
                    nc.vector.tensor_tensor(out=osl, in0=y_b, in1=xsl,
                                            op=ALU.mult)

        # ---------------- apply block ----------------
        SY_SX = [(sy, sx) for sy in range(3) for sx in range(3)]
        ft_full = featT5[:, :, :, :]

        def apply_block(bi, y0, rows, vpool, ppool, och, vpsum):
            vt = vpool.tile([128, 3, 3, CIN, rows], BF16, tag=f"vt{rows}")
            A = ppool.tile([128, 3, 3, CIN, rows], BF16, tag=f"A{rows}")
            vb = val_blk[:, bi % 2, :, 0:rows, :]

            def dve_mult(dst, sy, sx):
                for kx in range(3):
                    foff = (ft_full.offset + (sx + kx) * SH + (y0 + sy + 1))
                    fsl = bass.AP(tensor=ft_full.tensor, offset=foff,
                                  ap=[ft_full.ap[0], [1, 3], [NR, CIN],
                                      [1, rows]])
                    msl0 = m2[:, sy, sx, kx, :, y0 : y0 + rows]
                    msl = bass.AP(tensor=msl0.tensor, offset=msl0.offset,
                                  ap=[msl0.ap[0], msl0.ap[1], [0, CIN],
                                      msl0.ap[2]])
                    nc.vector.tensor_tensor(out=dst[:, kx, :, :, :],
                                            in0=fsl, in1=msl, op=ALU.mult)

            sy, sx = SY_SX[0]
            dve_mult(vt, sy, sx)
            for sy, sx in SY_SX[1:]:
                dve_mult(A, sy, sx)
                nc.vector.tensor_tensor(out=vt[:, :, :, :, :],
                                        in0=vt[:, :, :, :, :],
                                        in1=A[:, :, :, :, :], op=ALU.add)

            # back-transpose vt -> val_blk [(c, p-pair); t, y, x]
            for t in range(5):
                pst = vpsum.tile([128, rows * 128], BF16, tag="bt")
                for pp in range(2):
                    p = 2 * t + pp
                    if p >= KK:
                        continue
                    ky, kx = p // 3, p % 3
                    for j in range(rows):
                        nc.tensor.transpose(
                            pst[64 * pp : 64 * pp + 64, 128 * j : 128 * (j + 1)],
                            vt[:, kx, ky, :, j], identb[:, :])
                hi = 128 if t < 4 else 64
                nc.scalar.copy(vb[0:hi, t, :, :], pst[0:hi, :])

            oc = och.tile([COUT, 16, W], F32, tag="oc")
            for c2 in range(rows // 4):
                ps = vpsum.tile([COUT, 512], F32, tag="mm")
                for t in range(5):
                    nc.tensor.matmul(ps[:, :], w2b[:, t, :],
                                     vb[:, t, 4 * c2 : 4 * c2 + 4, :],
                                     start=(t == 0), stop=(t == 4))
                nc.scalar.activation(oc[:, 4 * c2 : 4 * c2 + 4, :], ps[:, :],
                                     ACTF.Identity, bias=bias2c[:, :])
            nc.sync.dma_start(out=d_out[:, y0 : y0 + rows, :],
                              in_=oc[:, 0:rows, :])

        # ---------------- schedule ----------------
        with tc.tile_pool(name="ompsum", bufs=2, space="PSUM") as ompsum:
            for r0, r1 in [(0, 18), (18, 34), (34, 50), (50, NRI)]:
                nc.gpsimd.dma_start(out=interb2[:, r0:r1, :],
                                    in_=d_inter2[:, r0:r1, :])
            om_quarter(0, ompsum, interb2, om_sb)
            mask_build(0)
            om_quarter(1, ompsum, interb2, om_sb)
        omsb_cm.__exit__(None, None, None)

        with (
            tc.tile_pool(name="vpool", bufs=2) as vpool,
            tc.tile_pool(name="ppool", bufs=1) as ppool,
            tc.tile_pool(name="och", bufs=2) as och,
            tc.tile_pool(name="vpsum", bufs=2, space="PSUM") as vpsum,
        ):
            for bi, (y0, rows) in enumerate(BLOCKS):
                apply_block(bi, y0, rows, vpool, ppool, och, vpsum)
                if bi == 1:
                    mask_build(1)

    nc.compile()
    return nc


def _host_prep(inputs):
    """Build the 8 per-core input maps (numpy marshalling only)."""
    bf = ml_dtypes.bfloat16
    feat = np.ascontiguousarray(inputs["input_feat"], dtype=np.float32)
    inter = np.ascontiguousarray(inputs["inter"], dtype=np.float32)
    fea = np.asarray(inputs["fea"], dtype=np.float32)[:, :, 0, 0]  # [B, 256]
    weight = np.asarray(inputs["weight"], dtype=np.float32)
    bias = np.asarray(inputs["bias"], dtype=np.float32)
    com_w = np.asarray(inputs["com_w"], dtype=np.float32)
    com_b = np.asarray(inputs["com_b"], dtype=np.float32)
    c1_w = np.asarray(inputs["c1_w"], dtype=np.float32)
    c2_w = np.asarray(inputs["c2_w"], dtype=np.float32)

    # fold c2 into the static weight:  weight2[o2, c, p] (parameter prep)
    w_r = weight.reshape(COUT, CIN, KK)
    weight2 = np.einsum("ao,ocp->acp", c2_w, w_r)  # [64, 64, 9]
    w2 = np.zeros((128, 5, 64), np.float32)  # [(c, p-pair), ktile, o2]
    for p in range(KK):
        t, pp = p // 2, p % 2
        w2[64 * pp : 64 * pp + 64, t, :] = weight2[:, :, p].T  # [c, o2]
    w2 = w2.astype(bf)
    bias2 = (c2_w @ bias).reshape(COUT, 1)

    # com_w reordered: channels [dy x9, dx x9, sig x9]; layout [cin, tap, 27]
    perm = list(range(0, 18, 2)) + list(range(1, 18, 2)) + list(range(18, 27))
    comw = np.ascontiguousarray(
        com_w[perm].reshape(27, CIN, KK).transpose(1, 2, 0))  # [CIN, KK, 27]
    comb = com_b[perm].reshape(27, 1).astype(np.float32)
    # tap pairing for om conv: kx=0/1 stacked in partition halves, kx=2 single
    comw2 = np.zeros((128, 3, 27), np.float32)
    comw1 = np.zeros((CIN, 3, 27), np.float32)
    for dy in range(3):
        comw2[0:CIN, dy] = comw[:, 3 * dy + 0]
        comw2[CIN:128, dy] = comw[:, 3 * dy + 1]
        comw1[:, dy] = comw[:, 3 * dy + 2]
    comw2 = comw2.astype(bf)
    comw1 = comw1.astype(bf)

    # c1w duplicated over output cols so the fvec matmul fills 128 partitions
    c1w = np.ascontiguousarray(c1_w.T.reshape(2, 128, COUT).transpose(1, 0, 2))
    c1w2 = np.concatenate([c1w, c1w], axis=2).astype(bf)  # [128, 2, 128]
    ident = np.eye(128, dtype=np.float32).astype(bf)

    in_maps = []
    for i in range(8):
        b, h = i // 2, i % 2
        r0 = NOUT * h
        # host-transposed feat: [x(132, padded +-2), c, row]
        fpadT = np.zeros((132, CIN, NR), bf)
        glo, ghi = r0 - 3, r0 - 3 + NR
        slo, shi = max(0, glo), min(H, ghi)
        fpadT[2 : 2 + W, :, slo - glo : shi - glo] = (
            feat[b, :, slo:shi, :].astype(bf).transpose(2, 0, 1))
        ipad2 = np.zeros((128, NRI, 130), np.float32)
        glo, ghi = r0 - 1, r0 - 1 + NRI
        slo, shi = max(0, glo), min(H, ghi)
        ipad2[0:CIN, slo - glo : shi - glo, 1 : 1 + W] = inter[b, :, slo:shi, :]
        ipad2[CIN:128, :, 0:129] = ipad2[0:CIN, :, 1:130]
        ipad2 = ipad2.astype(bf)
        feac = np.ascontiguousarray(fea[b].reshape(2, 128).T).astype(bf)
        in_maps.append(dict(featT=fpadT, inter2=ipad2, w2=w2, comw2=comw2,
                            comw1=comw1, comb=comb, c1w=c1w2, fea=feac,
                            bias2=bias2, ident=ident))
    return in_maps


def kernel(**inputs) -> np.ndarray:
    if "nc" not in _CACHED:
        _CACHED["nc"] = _build_nc()
    nc = _CACHED["nc"]
    in_maps = _host_prep(inputs)
    res = run_bass_kernel_spmd(nc, in_maps, core_ids=list(range(8)),
                               **_CACHED.get("run_kwargs", {}))
    _CACHED["last_result"] = res
    out = np.zeros((B, COUT, H, W), np.float32)
    for i in range(8):
        b, h = i // 2, i % 2
        out[b, :, NOUT * h : NOUT * (h + 1), :] = res.results[i]["out"]
    return out


# revision 12
# speedup vs baseline: 1.1068x; 1.0644x over previous
"""Trainium2 Bass kernel for the DCN (modulated deformable conv) layer.

Self-contained: hardcodes all shapes. Shards data-parallel over (batch b x
row-half h) onto 8 NeuronCores; each core computes a [64, 64, 128] slab of
the [4, 64, 128, 128] output.

v4 over v3:
 - om conv + mask build + apply interleaved at 16-row quarters: DVE starts
   mask(0,16) as soon as the first quarter of the om conv lands, and apply
   block 0 right after -- removes the ~36us DVE wait on the full om conv.
 - last 16 output rows run as two fully-pipelined 8-row blocks (their own
   back-transpose + matmul + store), halving the exposed tail.
 - val_blk is a manually rotated persistent double buffer; its tap-9
   upper-half zero fill happens once at init on the gpsimd queue.
v3 over v2:
 - featT5 (x-partitioned, 5 x-shift slots) built on the HOST, loaded as 5
   direct slot DMAs from a [132, 64, 69] bf16 DRAM tensor.
 - om conv emitted in row-halves; bulk inputs land as bf16; fvec applied
   to the static weight via one ACT-engine per-partition scale.
"""

import numpy as np
import ml_dtypes
from contextlib import ExitStack

import concourse.bass as bass
import concourse.bacc as bacc
import concourse.tile as tile
from concourse import mybir
from concourse.bass_utils import run_bass_kernel_spmd

F32 = mybir.dt.float32
BF16 = mybir.dt.bfloat16
ALU = mybir.AluOpType
ACTF = mybir.ActivationFunctionType

B, CIN, COUT, H, W, K = 4, 64, 64, 128, 128, 3
KK = K * K
NOUT = 64          # out rows per core
NR = 69            # feat rows resident per core (row idx = y + ky + sy + 1)
NRI = 66           # inter rows resident (conv halo 1)
QR = 32            # om/mask half rows
CLAMP = 0.999999
SH = CIN * NR      # featT5 shift-slot stride (elements)
BLOCKS = [(0, 16), (16, 16), (32, 16), (48, 8), (56, 8)]

_CACHED = {}


def _build_nc():
    nc = bacc.Bacc("TRN2", target_bir_lowering=False)

    # ---- DRAM I/O (per-core views; same program on all 8 cores) ----
    d_featT = nc.dram_tensor("featT", [132, CIN, NR], BF16, kind="ExternalInput")
    d_inter2 = nc.dram_tensor("inter2", [128, NRI, 130], BF16, kind="ExternalInput")
    d_w2 = nc.dram_tensor("w2", [128, 5, 64], BF16, kind="ExternalInput")
    d_comw2 = nc.dram_tensor("comw2", [128, 3, 27], BF16, kind="ExternalInput")
    d_comw1 = nc.dram_tensor("comw1", [CIN, 3, 27], BF16, kind="ExternalInput")
    d_comb = nc.dram_tensor("comb", [27, 1], F32, kind="ExternalInput")
    d_c1w = nc.dram_tensor("c1w", [128, 2, 128], BF16, kind="ExternalInput")
    d_fea = nc.dram_tensor("fea", [128, 2], BF16, kind="ExternalInput")
    d_bias2 = nc.dram_tensor("bias2", [COUT, 1], F32, kind="ExternalInput")
    d_ident = nc.dram_tensor("ident", [128, 128], BF16, kind="ExternalInput")
    d_out = nc.dram_tensor("out", [COUT, NOUT, W], F32, kind="ExternalOutput")

    with ExitStack() as ctx:
        tc = ctx.enter_context(tile.TileContext(nc))

        # ---------------- persistent pool ----------------
        pers = ctx.enter_context(tc.tile_pool(name="pers", bufs=1))
        identb = pers.tile([128, 128], BF16)
        w2b = pers.tile([128, 5, 64], BF16)
        bias2c = pers.tile([COUT, 1], F32)
        omT = pers.tile([128, NOUT, 27], BF16)
        wys = pers.tile([128, 3, 3, 3, NOUT], BF16)   # [x; sy, kx, ky, y]
        wxs = pers.tile([128, 3, 3, 3, NOUT], BF16)   # [x; sx, kx, ky, y]
        m2 = pers.tile([128, 3, 3, 3, 3, NOUT], BF16)  # [x; sy, sx, kx, ky, y]
        featT5 = pers.tile([128, 5, CIN, NR], BF16)    # [x; shift, c, row]
        # om/mask per-quarter scratch
        mbig = pers.tile([128, 8, KK, QR], BF16)
        wmT = pers.tile([128, KK, QR], BF16, tag="wm")
        w0T = pers.tile([128, KK, QR], BF16, tag="w0")
        wpT = pers.tile([128, KK, QR], BF16, tag="wp")
        # apply: manually rotated val_blk double buffer
        val_blk = pers.tile([128, 2, 5, 16, W], BF16)

        # ---------------- init loads ----------------
        omsb_cm = tc.tile_pool(name="omsb", bufs=1)
        omsb = omsb_cm.__enter__()
        comwb2 = omsb.tile([128, 3, 27], BF16)
        comwb1 = omsb.tile([CIN, 3, 27], BF16)
        combc = omsb.tile([27, 1], F32)
        c1wb = omsb.tile([128, 2, 128], BF16)
        feab = omsb.tile([128, 2], BF16)
        fvec = omsb.tile([128, 1], F32)
        w2raw = omsb.tile([128, 5, 64], BF16)
        interb2 = omsb.tile([128, NRI, 130], BF16)
        om_sb = omsb.tile([27, QR, W], BF16)
        nc.sync.dma_start(out=interb2[:, 0:12, :], in_=d_inter2[:, 0:12, :])
        nc.scalar.dma_start(out=comwb2, in_=d_comw2[:, :, :])
        nc.scalar.dma_start(out=comwb1, in_=d_comw1[:, :, :])
        nc.scalar.dma_start(out=combc, in_=d_comb[:, :])
        nc.sync.dma_start(out=identb, in_=d_ident[:, :])
        nc.sync.dma_start(out=interb2[:, 12:34, :], in_=d_inter2[:, 12:34, :])
        nc.scalar.dma_start(out=c1wb, in_=d_c1w[:, :, :])
        nc.scalar.dma_start(out=feab, in_=d_fea[:, :])
        nc.scalar.dma_start(out=bias2c, in_=d_bias2[:, :])
        nc.scalar.dma_start(out=w2raw, in_=d_w2[:, :, :])
        for s, q in zip(range(5), [nc.sync, nc.sync, nc.scalar, nc.scalar,
                                   nc.sync]):
            q.dma_start(out=featT5[:, s, :, :], in_=d_featT[s : s + 128, :, :])
        for r0, r1 in [(34, 50), (50, NRI)]:
            nc.gpsimd.dma_start(out=interb2[:, r0:r1, :],
                                in_=d_inter2[:, r0:r1, :])
        # zero tap-9 upper half of both val_blk buffers once
        nc.gpsimd.memset(val_blk[64:128, :, 4, :, :], 0.0)

        # PE p-state warmup: keep the array busy while the inter DMA lands
        with tc.tile_pool(name="warm", bufs=1, space="PSUM") as warmp:
            ps_w = warmp.tile([128, 128], F32)
            for _ in range(40):
                nc.tensor.matmul(ps_w[:, :], identb[:, :], identb[:, :],
                                 start=True, stop=True)

        with tc.tile_pool(name="psum_sm", bufs=1, space="PSUM") as psum_sm:
            # fvec = c1_w @ fea -> [128, 1] (host duplicated cols -> both halves)
            ps_fv = psum_sm.tile([128, 1], F32)
            for k in range(2):
                nc.tensor.matmul(ps_fv[:, :], c1wb[:, k, :], feab[:, k : k + 1],
                                 start=(k == 0), stop=(k == 1))
            nc.scalar.copy(fvec[:, :], ps_fv[:, :])
            # w2b = w2raw * fvec[c]  (ACT per-partition scale, bf16)
            nc.scalar.activation(w2b.rearrange("p a b -> p (a b)")[:, :],
                                 w2raw.rearrange("p a b -> p (a b)")[:, :],
                                 ACTF.Copy, scale=fvec[:, :])

        # ---------------- om conv quarter + mask build ----------------
        def om_quarter(q, ompsum, interb2, om_sb):
            yq = QR * q
            for n in range(QR // 4):  # 512-wide chunks = 4 rows
                ps = ompsum.tile([27, 512], F32)
                yl = 4 * n
                for dy in range(3):
                    mv_pair = interb2[:, yq + yl + dy : yq + yl + dy + 4, 0:W]
                    nc.tensor.matmul(ps[:, :], comwb2[:, dy, :], mv_pair,
                                     start=(dy == 0), stop=False)
                    mv_sing = interb2[0:CIN, yq + yl + dy : yq + yl + dy + 4,
                                      2 : 2 + W]
                    nc.tensor.matmul(ps[:, :], comwb1[:, dy, :], mv_sing,
                                     start=False, stop=(dy == 2))
                nc.scalar.activation(
                    om_sb[:, yl : yl + 4, :].rearrange("p a b -> p (a b)"),
                    ps[:, :], ACTF.Identity, bias=combc[:, :])
            for g in range(QR // 16):
                pst = ompsum.tile([128, 16, 28], BF16)
                for j in range(16):
                    nc.tensor.transpose(pst[:, j, 0:27],
                                        om_sb[:, 16 * g + j, :],
                                        identb[0:27, 0:27])
                nc.scalar.copy(omT[:, yq + 16 * g : yq + 16 * g + 16, :],
                               pst[:, :, 0:27])

        def mask_build(q):
            h0, h1 = QR * q, QR * q + QR
            dyT, dxT, sgT, ey, ly, ay, f0, s = [
                mbig[:, i, :, :] for i in range(8)]
            # repack from omT (ch-minor) into [128, p, y] (ACT)
            for dst, lo in [(dyT, 0), (dxT, 9), (sgT, 18)]:
                nc.scalar.copy(dst,
                               omT[:, h0:h1, lo : lo + 9].rearrange(
                                   "p y c -> p c y"))
            nc.scalar.activation(sgT, sgT, ACTF.Sigmoid)

            def wslot_ap(wt, si):
                sl = wt[:, si, :, :, h0:h1]
                return bass.AP(tensor=sl.tensor, offset=sl.offset,
                               ap=[sl.ap[0], [64, 3], [192, 3], [1, QR]])

            def pmaj(sl):
                """Reorder a [128, 9(p), QR] scratch view to (ky, kx, y)."""
                return bass.AP(tensor=sl.tensor, offset=sl.offset,
                               ap=[sl.ap[0], [3 * QR, 3], [QR, 3], [1, QR]])

            for dT, wt, fold_sig in [(dyT, wys, True), (dxT, wxs, False)]:
                nc.vector.tensor_scalar(out=dT, in0=dT,
                                        scalar1=-CLAMP, scalar2=CLAMP,
                                        op0=ALU.max, op1=ALU.min)
                nc.vector.tensor_scalar(out=ey, in0=dT, scalar1=0.0,
                                        scalar2=None, op0=ALU.is_lt)
                nc.vector.tensor_tensor(out=ly, in0=dT, in1=ey, op=ALU.add)
                nc.vector.tensor_scalar(out=ay, in0=ly, scalar1=-1.0,
                                        scalar2=1.0, op0=ALU.mult, op1=ALU.add)
                nc.vector.tensor_scalar(out=f0, in0=ey, scalar1=-1.0,
                                        scalar2=1.0, op0=ALU.mult, op1=ALU.add)
                nc.vector.tensor_tensor(out=wmT[:, :, :], in0=ey, in1=ay,
                                        op=ALU.mult)
                nc.vector.tensor_tensor(out=wpT[:, :, :], in0=f0, in1=ly,
                                        op=ALU.mult)
                nc.vector.tensor_tensor(out=s, in0=wmT[:, :, :],
                                        in1=wpT[:, :, :], op=ALU.add)
                nc.vector.tensor_scalar(out=w0T[:, :, :], in0=s, scalar1=-1.0,
                                        scalar2=1.0, op0=ALU.mult, op1=ALU.add)
                for si, wk in enumerate([wmT, w0T, wpT]):
                    if fold_sig:
                        nc.vector.tensor_tensor(out=wslot_ap(wt, si),
                                                in0=pmaj(wk[:, :, :]),
                                                in1=pmaj(sgT), op=ALU.mult)
                    else:
                        nc.scalar.copy(wslot_ap(wt, si), pmaj(wk[:, :, :]))

            # m2[x; sy, sx, kx, ky, y] = wys[sy] * wxs[sx] (bf16, 2x)
            for sy in range(3):
                for sx in range(3):
                    osl = m2[:, sy, sx, :, :, h0:h1]
                    ysl = wys[:, sy, :, :, h0:h1]
                    y_b = bass.AP(tensor=ysl.tensor, offset=ysl.offset,
                                  ap=[ysl.ap[0], [192, 3], [64, 3], [1, QR]])
                    xsl = wxs[:, sx, :, :, h0:h1]
                    nc.vector.tensor_tensor(out=osl, in0=y_b, in1=xsl,
                                            op=ALU.mult)

        # ---------------- apply block ----------------
        SY_SX = [(sy, sx) for sy in range(3) for sx in range(3)]
        ft_full = featT5[:, :, :, :]

        def apply_block(bi, y0, rows, vpool, ppool, och, vpsum):
            vt = vpool.tile([128, 3, 3, CIN, rows], BF16, tag=f"vt{rows}")
            A = ppool.tile([128, 3, 3, CIN, rows], BF16, tag=f"A{rows}")
            vb = val_blk[:, bi % 2, :, 0:rows, :]

            def dve_mult(dst, sy, sx):
                for kx in range(3):
                    foff = (ft_full.offset + (sx + kx) * SH + (y0 + sy + 1))
                    fsl = bass.AP(tensor=ft_full.tensor, offset=foff,
                                  ap=[ft_full.ap[0], [1, 3], [NR, CIN],
                                      [1, rows]])
                    msl0 = m2[:, sy, sx, kx, :, y0 : y0 + rows]
                    msl = bass.AP(tensor=msl0.tensor, offset=msl0.offset,
                                  ap=[msl0.ap[0], msl0.ap[1], [0, CIN],
                                      msl0.ap[2]])
                    nc.vector.tensor_tensor(out=dst[:, kx, :, :, :],
                                            in0=fsl, in1=msl, op=ALU.mult)

            sy, sx = SY_SX[0]
            dve_mult(vt, sy, sx)
            for sy, sx in SY_SX[1:]:
                dve_mult(A, sy, sx)
                nc.vector.tensor_tensor(out=vt[:, :, :, :, :],
                                        in0=vt[:, :, :, :, :],
                                        in1=A[:, :, :, :, :], op=ALU.add)

            # back-transpose vt -> val_blk [(c, p-pair); t, y, x]
            for t in range(5):
                pst = vpsum.tile([128, rows * 128], BF16, tag="bt")
                for pp in range(2):
                    p = 2 * t + pp
                    if p >= KK:
                        continue
                    ky, kx = p // 3, p % 3
                    for j in range(rows):
                        nc.tensor.transpose(
                            pst[64 * pp : 64 * pp + 64, 128 * j : 128 * (j + 1)],
                            vt[:, kx, ky, :, j], identb[:, :])
                hi = 128 if t < 4 else 64
                nc.scalar.copy(vb[0:hi, t, :, :], pst[0:hi, :])

            oc = och.tile([COUT, 16, W], F32, tag="oc")
            for c2 in range(rows // 4):
                ps = vpsum.tile([COUT, 512], F32, tag="mm")
                for t in range(5):
                    nc.tensor.matmul(ps[:, :], w2b[:, t, :],
                                     vb[:, t, 4 * c2 : 4 * c2 + 4, :],
                                     start=(t == 0), stop=(t == 4))
                nc.scalar.activation(oc[:, 4 * c2 : 4 * c2 + 4, :], ps[:, :],
                                     ACTF.Identity, bias=bias2c[:, :])
            nc.sync.dma_start(out=d_out[:, y0 : y0 + rows, :],
                              in_=oc[:, 0:rows, :])

        # ---------------- schedule ----------------
        with tc.tile_pool(name="ompsum", bufs=4, space="PSUM") as ompsum:
            om_quarter(0, ompsum, interb2, om_sb)
            mask_build(0)
            om_quarter(1, ompsum, interb2, om_sb)
        omsb_cm.__exit__(None, None, None)

        with (
            tc.tile_pool(name="vpool", bufs=2) as vpool,
            tc.tile_pool(name="ppool", bufs=1) as ppool,
            tc.tile_pool(name="och", bufs=2) as och,
            tc.tile_pool(name="vpsum", bufs=2, space="PSUM") as vpsum,
        ):
            for bi, (y0, rows) in enumerate(BLOCKS):
                apply_block(bi, y0, rows, vpool, ppool, och, vpsum)
                if bi == 1:
                    mask_build(1)

    nc.compile()
    return nc


def _host_prep(inputs):
    """Build the 8 per-core input maps (numpy marshalling only)."""
    bf = ml_dtypes.bfloat16
    feat = np.ascontiguousarray(inputs["input_feat"], dtype=np.float32)
    inter = np.ascontiguousarray(inputs["inter"], dtype=np.float32)
    fea = np.asarray(inputs["fea"], dtype=np.float32)[:, :, 0, 0]  # [B, 256]
    weight = np.asarray(inputs["weight"], dtype=np.float32)
    bias = np.asarray(inputs["bias"], dtype=np.float32)
    com_w = np.asarray(inputs["com_w"], dtype=np.float32)
    com_b = np.asarray(inputs["com_b"], dtype=np.float32)
    c1_w = np.asarray(inputs["c1_w"], dtype=np.float32)
    c2_w = np.asarray(inputs["c2_w"], dtype=np.float32)

    # fold c2 into the static weight:  weight2[o2, c, p] (parameter prep)
    w_r = weight.reshape(COUT, CIN, KK)
    weight2 = np.einsum("ao,ocp->acp", c2_w, w_r)  # [64, 64, 9]
    w2 = np.zeros((128, 5, 64), np.float32)  # [(c, p-pair), ktile, o2]
    for p in range(KK):
        t, pp = p // 2, p % 2
        w2[64 * pp : 64 * pp + 64, t, :] = weight2[:, :, p].T  # [c, o2]
    w2 = w2.astype(bf)
    bias2 = (c2_w @ bias).reshape(COUT, 1)

    # com_w reordered: channels [dy x9, dx x9, sig x9]; layout [cin, tap, 27]
    perm = list(range(0, 18, 2)) + list(range(1, 18, 2)) + list(range(18, 27))
    comw = np.ascontiguousarray(
        com_w[perm].reshape(27, CIN, KK).transpose(1, 2, 0))  # [CIN, KK, 27]
    comb = com_b[perm].reshape(27, 1).astype(np.float32)
    # tap pairing for om conv: kx=0/1 stacked in partition halves, kx=2 single
    comw2 = np.zeros((128, 3, 27), np.float32)
    comw1 = np.zeros((CIN, 3, 27), np.float32)
    for dy in range(3):
        comw2[0:CIN, dy] = comw[:, 3 * dy + 0]
        comw2[CIN:128, dy] = comw[:, 3 * dy + 1]
        comw1[:, dy] = comw[:, 3 * dy + 2]
    comw2 = comw2.astype(bf)
    comw1 = comw1.astype(bf)

    # c1w duplicated over output cols so the fvec matmul fills 128 partitions
    c1w = np.ascontiguousarray(c1_w.T.reshape(2, 128, COUT).transpose(1, 0, 2))
    c1w2 = np.concatenate([c1w, c1w], axis=2).astype(bf)  # [128, 2, 128]
    ident = np.eye(128, dtype=np.float32).astype(bf)

    in_maps = []
    for i in range(8):
        b, h = i // 2, i % 2
        r0 = NOUT * h
        # host-transposed feat: [x(132, padded +-2), c, row]
        fpadT = np.zeros((132, CIN, NR), bf)
        glo, ghi = r0 - 3, r0 - 3 + NR
        slo, shi = max(0, glo), min(H, ghi)
        fpadT[2 : 2 + W, :, slo - glo : shi - glo] = (
            feat[b, :, slo:shi, :].astype(bf).transpose(2, 0, 1))
        ipad2 = np.zeros((128, NRI, 130), np.float32)
        glo, ghi = r0 - 1, r0 - 1 + NRI
        slo, shi = max(0, glo), min(H, ghi)
        ipad2[0:CIN, slo - glo : shi - glo, 1 : 1 + W] = inter[b, :, slo:shi, :]
        ipad2[CIN:128, :, 0:129] = ipad2[0:CIN, :, 1:130]
        ipad2 = ipad2.astype(bf)
        feac = np.ascontiguousarray(fea[b].reshape(2, 128).T).astype(bf)
        in_maps.append(dict(featT=fpadT, inter2=ipad2, w2=w2, comw2=comw2,
                            comw1=comw1, comb=comb, c1w=c1w2, fea=feac,
                            bias2=bias2, ident=ident))
    return in_maps


def kernel(**inputs) -> np.ndarray:
    if "nc" not in _CACHED:
        _CACHED["nc"] = _build_nc()
    nc = _CACHED["nc"]
    in_maps = _host_prep(inputs)
    res = run_bass_kernel_spmd(nc, in_maps, core_ids=list(range(8)),
                               **_CACHED.get("run_kwargs", {}))
    _CACHED["last_result"] = res
    out = np.zeros((B, COUT, H, W), np.float32)
    for i in range(8):
        b, h = i // 2, i % 2
        out[b, :, NOUT * h : NOUT * (h + 1), :] = res.results[i]["out"]
    return out
